# revision 8
# baseline (speedup 1.0000x reference)
"""Trainium2 Bass kernel for nn_AttentionHelper (sparse_attention).

Math (per batch b):
    energy[l,m] = sum_c Q[c,l] K[c,m] / sqrt(C)
    logits      = energy + log(mask[m] + 1e-9)
    att         = softmax_m(logits) * mask[m]
    out[c,l]    = sum_m V[c,m] att[l,m]

Strategy: data-parallel over batch B=16 across 8 NeuronCores (2 batches per
core), full LxL attention per batch on one core, no collectives.

Per-core kernel (per batch, per 512-wide slice of l):
  1. E^T[m,l] = K^T Q via PE matmuls (fp32r, contraction over C in 2 chunks
     of 128), m on partitions so the softmax mask fold is a per-partition
     activation bias.
  2. n1[m,l] = exp(E^T/16 + log(mask[m]+1e-9)) -- single fused ACT per
     m-chunk (scale + per-partition bias).
  3. d[l] = sum_m n1[m,l]: DVE pairwise tree over the 16 m-chunk tiles,
     then a [128,1]-ones matmul for the in-chunk partition reduction.
  4. out[c,l] = sum_m (V[c,m]*mask[m]) n1[m,l] / d[l]: PE matmuls with
     lhsT = (V^T * mask) built once per batch via PE-transpose; division by
     d via reciprocal + DMA row-broadcast + DVE multiply.
"""

import numpy as np

import concourse.bacc as bacc
import concourse.bass as bass
import concourse.tile as tile
from concourse import mybir
from concourse.bass_utils import run_bass_kernel_spmd
from concourse.masks import make_identity

B, C, L = 16, 256, 2048
NCORES = 8
BS = B // NCORES  # batches per core
P = 128
CCH = C // P      # 2 chunks over channels
MCH = L // P      # 16 chunks over m (key positions)
NQ = 4            # process l in 4 quarters
LQ = L // NQ      # 512 = matmul moving free dim
F32 = mybir.dt.float32
F32R = mybir.dt.float32r
EXP = mybir.ActivationFunctionType.Exp
LOG = mybir.ActivationFunctionType.Ln


def _emit(ctx, tc, q_d, k_d, v_d, m_d, o_d):
    nc = tc.nc

    const_pool = ctx.enter_context(tc.tile_pool(name="const", bufs=1))
    qk_pool = ctx.enter_context(tc.tile_pool(name="qk", bufs=1))
    stage_pool = ctx.enter_context(tc.tile_pool(name="stage", bufs=2))
    vt_pool = ctx.enter_context(tc.tile_pool(name="vt", bufs=1))
    vblk_pool = ctx.enter_context(tc.tile_pool(name="vblk", bufs=4))
    mask_pool = ctx.enter_context(tc.tile_pool(name="mask", bufs=2))
    n1_pool = ctx.enter_context(tc.tile_pool(name="n1", bufs=2))
    ts_pool = ctx.enter_context(tc.tile_pool(name="ts", bufs=2))
    out_pool = ctx.enter_context(tc.tile_pool(name="outp", bufs=4))
    rd_pool = ctx.enter_context(tc.tile_pool(name="rd", bufs=2))
    ps_e = ctx.enter_context(tc.tile_pool(name="ps_e", bufs=4, space="PSUM"))
    ps_o = ctx.enter_context(tc.tile_pool(name="ps_o", bufs=2, space="PSUM"))
    ps_misc = ctx.enter_context(tc.tile_pool(name="ps_m", bufs=2, space="PSUM"))

    identity = const_pool.tile([P, P], F32, name="identity")
    make_identity(nc, identity[:])
    ones_f = const_pool.tile([P, 1], F32, name="ones_f")
    nc.vector.memset(ones_f[:], 1.0)
    ones = const_pool.tile([P, 1], F32R, name="ones")
    nc.vector.tensor_copy(ones[:], ones_f[:])

    # per-batch persistent tiles, filled by prep(b)
    state = {}

    def prep(b):
        # mask prep: mask_pt[p, j] = mask[j*128 + p]
        mask_pt = mask_pool.tile([P, MCH], F32, tag="mask_pt", name=f"mask_pt{b}")
        nc.sync.dma_start(
            out=mask_pt[:], in_=m_d[b, 0, :].rearrange("(j p) -> p j", p=P)
        )
        w1 = mask_pool.tile([P, MCH], F32, tag="w1", name=f"w1_{b}")
        nc.vector.tensor_scalar_add(w1[:], mask_pt[:], 1e-9)
        logw1 = mask_pool.tile([P, MCH], F32, tag="logw1", name=f"logw1_{b}")
        nc.scalar.activation(logw1[:], w1[:], LOG)

        # Q/K resident per batch
        q_sb = []
        k_sb = []
        for src, dst, pfx in ((q_d, q_sb, "q"), (k_d, k_sb, "k")):
            for cc in range(CCH):
                stg = stage_pool.tile([P, L], F32, tag="stage", name=f"stg_{pfx}{b}_{cc}")
                nc.sync.dma_start(out=stg[:], in_=src[b, cc * P : (cc + 1) * P, :])
                t_ = qk_pool.tile([P, L], F32R, tag=f"{pfx}{cc}", name=f"{pfx}{b}_{cc}")
                # rounding copy to fp32r for the PE
                nc.vector.tensor_copy(t_[:], stg[:])
                dst.append(t_)

        # V^T * mask, built 128x128 block at a time via PE transpose
        vt = vt_pool.tile([P, MCH, C], F32R, tag="vt", name=f"vt{b}")
        for j in range(MCH):
            for cc in range(CCH):
                vblk = vblk_pool.tile([P, P], F32, tag="vblk", name=f"vb{b}_{j}_{cc}")
                nc.sync.dma_start(
                    out=vblk[:],
                    in_=v_d[b, cc * P : (cc + 1) * P, j * P : (j + 1) * P],
                )
                tr_ps = ps_misc.tile([P, P], F32, tag="misc", name=f"tr{b}_{j}_{cc}")
                nc.tensor.transpose(tr_ps[:], vblk[:], identity[:])
                nc.vector.tensor_scalar_mul(
                    vt[:, j, cc * P : (cc + 1) * P], tr_ps[:], mask_pt[:, j : j + 1]
                )
        state[b] = dict(q=q_sb, k=k_sb, vt=vt, logw1=logw1)

    def emit_qk_exp(b, qt):
        st = state[b]
        lq = qt * LQ
        n1 = n1_pool.tile([P, MCH, LQ], F32R, tag="n1", name=f"n1_{b}_{qt}")
        for j in range(MCH):
            e_ps = ps_e.tile([P, LQ], F32, tag="E", name=f"e_{b}_{qt}_{j}")
            for cc in range(CCH):
                nc.tensor.matmul(
                    e_ps[:],
                    lhsT=st["k"][cc][:, j * P : (j + 1) * P],
                    rhs=st["q"][cc][:, lq : lq + LQ],
                    start=(cc == 0),
                    stop=(cc == CCH - 1),
                )
            nc.scalar.activation(
                out=n1[:, j, :],
                in_=e_ps[:],
                func=EXP,
                bias=st["logw1"][:, j : j + 1],
                scale=1.0 / 16.0,
            )
        return n1

    def emit_tail(b, qt, n1):
        st = state[b]
        lq = qt * LQ

        # attention @ V first on PE (depends only on n1 + vt)
        o_ps = []
        for cg in range(CCH):
            ops = ps_o.tile([P, LQ], F32, tag="O", name=f"o_{b}_{qt}_{cg}")
            for j in range(MCH):
                nc.tensor.matmul(
                    ops[:],
                    lhsT=st["vt"][:, j, cg * P : (cg + 1) * P],
                    rhs=n1[:, j, :],
                    start=(j == 0),
                    stop=(j == MCH - 1),
                )
            o_ps.append(ops)

        # denominator: DVE tree over 16 m-chunks, then partition-reduce matmul
        accs = []
        for g in range(4):
            acc = ts_pool.tile([P, LQ], F32R, tag=f"ts{g}", name=f"ts{g}_{b}_{qt}")
            nc.vector.tensor_add(acc[:], n1[:, 4 * g, :], n1[:, 4 * g + 1, :])
            nc.vector.tensor_add(acc[:], acc[:], n1[:, 4 * g + 2, :])
            nc.vector.tensor_add(acc[:], acc[:], n1[:, 4 * g + 3, :])
            accs.append(acc)
        nc.vector.tensor_add(accs[0][:], accs[0][:], accs[1][:])
        nc.vector.tensor_add(accs[2][:], accs[2][:], accs[3][:])
        nc.vector.tensor_add(accs[0][:], accs[0][:], accs[2][:])

        d_ps = ps_misc.tile([1, LQ], F32, tag="misc", name=f"d_{b}_{qt}")
        nc.tensor.matmul(
            d_ps[:],
            lhsT=ones[:],
            rhs=accs[0][:],
            start=True,
            stop=True,
        )
        d_row = rd_pool.tile([1, LQ], F32, tag="d_row", name=f"dr_{b}_{qt}")
        nc.vector.tensor_copy(d_row[:], d_ps[:])
        r_row = rd_pool.tile([1, LQ], F32, tag="r_row", name=f"rr_{b}_{qt}")
        nc.vector.reciprocal_approx_fast(out=r_row[:], in_=d_row[:])
        rdbc = rd_pool.tile([P, LQ], F32, tag="rdbc", name=f"rb_{b}_{qt}")
        nc.gpsimd.partition_broadcast(rdbc[:], r_row[:])

        for cg in range(CCH):
            out_t = out_pool.tile([P, LQ], F32, tag="out", name=f"ot_{b}_{qt}_{cg}")
            nc.vector.tensor_mul(out_t[:], o_ps[cg][:], rdbc[:])
            nc.sync.dma_start(
                out=o_d[b, cg * P : (cg + 1) * P, lq : lq + LQ], in_=out_t[:]
            )

    # software-pipelined emission: AV(qt) goes to the PE queue after QK(qt+1)
    # so exp(qt) has a full QK-quarter of ACT time before PE needs n1(qt)
    work = []  # (b, qt, n1)
    prep(0)
    for b in range(BS):
        for qt in range(NQ):
            work.append((b, qt, emit_qk_exp(b, qt)))
            if len(work) > 1:
                emit_tail(*work.pop(0))
        if b + 1 < BS:
            prep(b + 1)
    while work:
        emit_tail(*work.pop(0))


def _build():
    nc = bacc.Bacc(
        "TRN2",
        target_bir_lowering=False,
        debug=False,
        enable_asserts=False,
        num_devices=NCORES,
    )
    q_d = nc.dram_tensor("proj_query", [BS, C, L], F32, kind="ExternalInput")
    k_d = nc.dram_tensor("proj_key", [BS, C, L], F32, kind="ExternalInput")
    v_d = nc.dram_tensor("proj_val", [BS, C, L], F32, kind="ExternalInput")
    m_d = nc.dram_tensor("padding_mask", [BS, 1, L], F32, kind="ExternalInput")
    o_d = nc.dram_tensor("out", [BS, C, L], F32, kind="ExternalOutput")

    from contextlib import ExitStack

    with tile.TileContext(nc) as tc:
        with ExitStack() as ctx:
            _emit(ctx, tc, q_d.ap(), k_d.ap(), v_d.ap(), m_d.ap(), o_d.ap())
    nc.compile()
    return nc


_cached_nc = None


def get_nc():
    global _cached_nc
    if _cached_nc is None:
        _cached_nc = _build()
    return _cached_nc


def make_in_maps(proj_query, proj_key, proj_val, padding_mask):
    q = np.ascontiguousarray(np.asarray(proj_query, dtype=np.float32))
    k = np.ascontiguousarray(np.asarray(proj_key, dtype=np.float32))
    v = np.ascontiguousarray(np.asarray(proj_val, dtype=np.float32))
    m = np.ascontiguousarray(np.asarray(padding_mask, dtype=np.float32))
    assert q.shape == (B, C, L) and m.shape == (B, 1, L)
    in_maps = []
    for i in range(NCORES):
        sl = slice(i * BS, (i + 1) * BS)
        in_maps.append(
            {
                "proj_query": np.ascontiguousarray(q[sl]),
                "proj_key": np.ascontiguousarray(k[sl]),
                "proj_val": np.ascontiguousarray(v[sl]),
                "padding_mask": np.ascontiguousarray(m[sl]),
            }
        )
    return in_maps


def kernel(proj_query, proj_key, proj_val, padding_mask):
    nc = get_nc()
    in_maps = make_in_maps(proj_query, proj_key, proj_val, padding_mask)
    res = run_bass_kernel_spmd(nc, in_maps, core_ids=list(range(NCORES)))
    return np.concatenate([res.results[i]["out"] for i in range(NCORES)], axis=0)


# revision 9
# speedup vs baseline: 1.3608x; 1.3608x over previous
"""Trainium2 Bass kernel for nn_AttentionHelper (sparse_attention).

Math (per batch b):
    energy[l,m] = sum_c Q[c,l] K[c,m] / sqrt(C)
    logits      = energy + log(mask[m] + 1e-9)
    att         = softmax_m(logits) * mask[m]
    out[c,l]    = sum_m V[c,m] att[l,m]

Strategy: data-parallel over batch B=16 across 8 NeuronCores (2 batches per
core), full LxL attention per batch on one core, no collectives.

Per-core kernel (per batch, per 512-wide slice of l):
  1. E^T[m,l] = K^T Q via PE matmuls (fp32r, contraction over C in 2 chunks
     of 128), m on partitions so the softmax mask fold is a per-partition
     activation bias.
  2. n1[m,l] = exp(E^T/16 + log(mask[m]+1e-9)) -- single fused ACT per
     m-chunk (scale + per-partition bias).
  3. d[l] = sum_m n1[m,l]: DVE pairwise tree over the 16 m-chunk tiles,
     then a [128,1]-ones matmul for the in-chunk partition reduction.
  4. out[c,l] = sum_m (V[c,m]*mask[m]) n1[m,l] / d[l]: PE matmuls with
     lhsT = (V^T * mask) built once per batch via PE-transpose; division by
     d via reciprocal + DMA row-broadcast + DVE multiply.
"""

import numpy as np

import concourse.bacc as bacc
import concourse.bass as bass
import concourse.tile as tile
from concourse import mybir
from concourse.bass_utils import run_bass_kernel_spmd
from concourse.masks import make_identity

B, C, L = 16, 256, 2048
NCORES = 8
BS = B // NCORES  # batches per core
P = 128
CCH = C // P      # 2 chunks over channels
MCH = L // P      # 16 chunks over m (key positions)
NQ = 4            # process l in 4 quarters
LQ = L // NQ      # 512 = matmul moving free dim
F32 = mybir.dt.float32
F32R = mybir.dt.float32r
BF16 = mybir.dt.bfloat16
import os as _os
MM_DT = {"f32r": F32R, "bf16": BF16}[_os.environ.get("BASS_MM_DT", "f32r")]
EXP = mybir.ActivationFunctionType.Exp
LOG = mybir.ActivationFunctionType.Ln


def _emit(ctx, tc, q_d, k_d, v_d, m_d, o_d):
    nc = tc.nc

    const_pool = ctx.enter_context(tc.tile_pool(name="const", bufs=1))
    qk_pool = ctx.enter_context(tc.tile_pool(name="qk", bufs=1))
    stage_pool = ctx.enter_context(tc.tile_pool(name="stage", bufs=2))
    vt_pool = ctx.enter_context(tc.tile_pool(name="vt", bufs=1))
    vblk_pool = ctx.enter_context(tc.tile_pool(name="vblk", bufs=4))
    mask_pool = ctx.enter_context(tc.tile_pool(name="mask", bufs=2))
    n1_pool = ctx.enter_context(tc.tile_pool(name="n1", bufs=2))
    ts_pool = ctx.enter_context(tc.tile_pool(name="ts", bufs=2))
    out_pool = ctx.enter_context(tc.tile_pool(name="outp", bufs=4))
    rd_pool = ctx.enter_context(tc.tile_pool(name="rd", bufs=2))
    ps_e = ctx.enter_context(tc.tile_pool(name="ps_e", bufs=4, space="PSUM"))
    ps_o = ctx.enter_context(tc.tile_pool(name="ps_o", bufs=2, space="PSUM"))
    ps_misc = ctx.enter_context(tc.tile_pool(name="ps_m", bufs=2, space="PSUM"))

    identity = const_pool.tile([P, P], F32, name="identity")
    make_identity(nc, identity[:])
    ones_f = const_pool.tile([P, 1], F32, name="ones_f")
    nc.vector.memset(ones_f[:], 1.0)
    ones = const_pool.tile([P, 1], MM_DT, name="ones")
    nc.vector.tensor_copy(ones[:], ones_f[:])

    # per-batch persistent tiles, filled by prep(b)
    state = {}

    def prep(b):
        # mask prep: mask_pt[p, j] = mask[j*128 + p]
        mask_pt = mask_pool.tile([P, MCH], F32, tag="mask_pt", name=f"mask_pt{b}")
        nc.sync.dma_start(
            out=mask_pt[:], in_=m_d[b, 0, :].rearrange("(j p) -> p j", p=P)
        )
        w1 = mask_pool.tile([P, MCH], F32, tag="w1", name=f"w1_{b}")
        nc.vector.tensor_scalar_add(w1[:], mask_pt[:], 1e-9)
        logw1 = mask_pool.tile([P, MCH], F32, tag="logw1", name=f"logw1_{b}")
        nc.scalar.activation(logw1[:], w1[:], LOG)

        # Q/K resident per batch
        q_sb = []
        k_sb = []
        for src, dst, pfx in ((q_d, q_sb, "q"), (k_d, k_sb, "k")):
            for cc in range(CCH):
                stg = stage_pool.tile([P, L], F32, tag="stage", name=f"stg_{pfx}{b}_{cc}")
                nc.sync.dma_start(out=stg[:], in_=src[b, cc * P : (cc + 1) * P, :])
                t_ = qk_pool.tile([P, L], MM_DT, tag=f"{pfx}{cc}", name=f"{pfx}{b}_{cc}")
                # rounding copy to fp32r for the PE
                nc.vector.tensor_copy(t_[:], stg[:])
                dst.append(t_)

        # V^T * mask, built 128x128 block at a time via PE transpose
        vt = vt_pool.tile([P, MCH, C], MM_DT, tag="vt", name=f"vt{b}")
        for j in range(MCH):
            for cc in range(CCH):
                vblk = vblk_pool.tile([P, P], F32, tag="vblk", name=f"vb{b}_{j}_{cc}")
                nc.sync.dma_start(
                    out=vblk[:],
                    in_=v_d[b, cc * P : (cc + 1) * P, j * P : (j + 1) * P],
                )
                tr_ps = ps_misc.tile([P, P], F32, tag="misc", name=f"tr{b}_{j}_{cc}")
                nc.tensor.transpose(tr_ps[:], vblk[:], identity[:])
                nc.vector.tensor_scalar_mul(
                    vt[:, j, cc * P : (cc + 1) * P], tr_ps[:], mask_pt[:, j : j + 1]
                )
        state[b] = dict(q=q_sb, k=k_sb, vt=vt, logw1=logw1)

    def emit_qk_exp(b, qt):
        st = state[b]
        lq = qt * LQ
        n1 = n1_pool.tile([P, MCH, LQ], MM_DT, tag="n1", name=f"n1_{b}_{qt}")
        for j in range(MCH):
            e_ps = ps_e.tile([P, LQ], F32, tag="E", name=f"e_{b}_{qt}_{j}")
            for cc in range(CCH):
                nc.tensor.matmul(
                    e_ps[:],
                    lhsT=st["k"][cc][:, j * P : (j + 1) * P],
                    rhs=st["q"][cc][:, lq : lq + LQ],
                    start=(cc == 0),
                    stop=(cc == CCH - 1),
                )
            nc.scalar.activation(
                out=n1[:, j, :],
                in_=e_ps[:],
                func=EXP,
                bias=st["logw1"][:, j : j + 1],
                scale=1.0 / 16.0,
            )
        return n1

    def emit_tail(b, qt, n1):
        st = state[b]
        lq = qt * LQ

        # attention @ V first on PE (depends only on n1 + vt)
        o_ps = []
        for cg in range(CCH):
            ops = ps_o.tile([P, LQ], F32, tag="O", name=f"o_{b}_{qt}_{cg}")
            for j in range(MCH):
                nc.tensor.matmul(
                    ops[:],
                    lhsT=st["vt"][:, j, cg * P : (cg + 1) * P],
                    rhs=n1[:, j, :],
                    start=(j == 0),
                    stop=(j == MCH - 1),
                )
            o_ps.append(ops)

        # denominator: DVE tree over 16 m-chunks, then partition-reduce matmul
        accs = []
        for g in range(4):
            acc = ts_pool.tile([P, LQ], MM_DT, tag=f"ts{g}", name=f"ts{g}_{b}_{qt}")
            nc.vector.tensor_add(acc[:], n1[:, 4 * g, :], n1[:, 4 * g + 1, :])
            nc.vector.tensor_add(acc[:], acc[:], n1[:, 4 * g + 2, :])
            nc.vector.tensor_add(acc[:], acc[:], n1[:, 4 * g + 3, :])
            accs.append(acc)
        nc.vector.tensor_add(accs[0][:], accs[0][:], accs[1][:])
        nc.vector.tensor_add(accs[2][:], accs[2][:], accs[3][:])
        nc.vector.tensor_add(accs[0][:], accs[0][:], accs[2][:])

        d_ps = ps_misc.tile([1, LQ], F32, tag="misc", name=f"d_{b}_{qt}")
        nc.tensor.matmul(
            d_ps[:],
            lhsT=ones[:],
            rhs=accs[0][:],
            start=True,
            stop=True,
        )
        d_row = rd_pool.tile([1, LQ], F32, tag="d_row", name=f"dr_{b}_{qt}")
        nc.vector.tensor_copy(d_row[:], d_ps[:])
        r_row = rd_pool.tile([1, LQ], F32, tag="r_row", name=f"rr_{b}_{qt}")
        nc.vector.reciprocal_approx_fast(out=r_row[:], in_=d_row[:])
        rdbc = rd_pool.tile([P, LQ], F32, tag="rdbc", name=f"rb_{b}_{qt}")
        nc.gpsimd.partition_broadcast(rdbc[:], r_row[:])

        for cg in range(CCH):
            out_t = out_pool.tile([P, LQ], F32, tag="out", name=f"ot_{b}_{qt}_{cg}")
            nc.vector.tensor_mul(out_t[:], o_ps[cg][:], rdbc[:])
            nc.sync.dma_start(
                out=o_d[b, cg * P : (cg + 1) * P, lq : lq + LQ], in_=out_t[:]
            )

    # software-pipelined emission: AV(qt) goes to the PE queue after QK(qt+1)
    # so exp(qt) has a full QK-quarter of ACT time before PE needs n1(qt)
    work = []  # (b, qt, n1)
    prep(0)
    for b in range(BS):
        for qt in range(NQ):
            work.append((b, qt, emit_qk_exp(b, qt)))
            if len(work) > 1:
                emit_tail(*work.pop(0))
        if b + 1 < BS:
            prep(b + 1)
    while work:
        emit_tail(*work.pop(0))


def _build():
    nc = bacc.Bacc(
        "TRN2",
        target_bir_lowering=False,
        debug=False,
        enable_asserts=False,
        num_devices=NCORES,
    )
    q_d = nc.dram_tensor("proj_query", [BS, C, L], F32, kind="ExternalInput")
    k_d = nc.dram_tensor("proj_key", [BS, C, L], F32, kind="ExternalInput")
    v_d = nc.dram_tensor("proj_val", [BS, C, L], F32, kind="ExternalInput")
    m_d = nc.dram_tensor("padding_mask", [BS, 1, L], F32, kind="ExternalInput")
    o_d = nc.dram_tensor("out", [BS, C, L], F32, kind="ExternalOutput")

    from contextlib import ExitStack

    with tile.TileContext(nc) as tc:
        with ExitStack() as ctx:
            _emit(ctx, tc, q_d.ap(), k_d.ap(), v_d.ap(), m_d.ap(), o_d.ap())
    nc.compile()
    return nc


_cached_nc = None


def get_nc():
    global _cached_nc
    if _cached_nc is None:
        _cached_nc = _build()
    return _cached_nc


def make_in_maps(proj_query, proj_key, proj_val, padding_mask):
    q = np.ascontiguousarray(np.asarray(proj_query, dtype=np.float32))
    k = np.ascontiguousarray(np.asarray(proj_key, dtype=np.float32))
    v = np.ascontiguousarray(np.asarray(proj_val, dtype=np.float32))
    m = np.ascontiguousarray(np.asarray(padding_mask, dtype=np.float32))
    assert q.shape == (B, C, L) and m.shape == (B, 1, L)
    in_maps = []
    for i in range(NCORES):
        sl = slice(i * BS, (i + 1) * BS)
        in_maps.append(
            {
                "proj_query": np.ascontiguousarray(q[sl]),
                "proj_key": np.ascontiguousarray(k[sl]),
                "proj_val": np.ascontiguousarray(v[sl]),
                "padding_mask": np.ascontiguousarray(m[sl]),
            }
        )
    return in_maps


def kernel(proj_query, proj_key, proj_val, padding_mask):
    nc = get_nc()
    in_maps = make_in_maps(proj_query, proj_key, proj_val, padding_mask)
    res = run_bass_kernel_spmd(nc, in_maps, core_ids=list(range(NCORES)))
    return np.concatenate([res.results[i]["out"] for i in range(NCORES)], axis=0)
